# revision 8
# baseline (speedup 1.0000x reference)
"""Trainium2 Bass kernel for nn_CP_Based (CP-decomposition interaction layer).

Math (full problem):
    t[b,f,r,u] = sum_d X[b,f,d] * K[d,r,f,u]      (B=1024, F=64, D=4, R=32, U=128)
    had[b,r,u] = prod_f t[b,f,r,u]
    out[b,u]   = sum_r had[b,r,u]

Strategy (v5):
  * Shard batch x units across 8 cores as (2 batch halves) x (4 unit
    quarters): per core B_loc=512 (4 partition tiles) and RU_loc = 32r x 32u
    = 1024 columns (u-major, r contiguous innermost for the final reduce).
  * Host-side feature grouping into multilinear factors: 16 triples (K=64,
    row-group pairs sharing a kt slot) + 4 quads (K=256 as 2 PSUM-accumulated
    K=128 passes) = 20 factor tiles per batch tile.  All matmul inputs fp16.
  * The 20 tiles are produced as 10 WIDE PAIRS [128, 2048] in a 2-slot PSUM
    ring (4 banks per slot).  Wide ops halve instruction count + semaphore
    traffic.
  * Consumption (a TensorTensor may read only ONE PSUM operand): 3 pairs
    feed a narrow fp32 fused chain G on DVE (copy + 5 muls, one PSUM
    operand each - precision anchor, no 16-bit quantization); 7 pairs
    drain via wide Act ACTIVATE to bf16.  Pool chains 3 of those wides
    (2 muls) + the final narrow merge; DVE chains the other 4 (3 wide
    bf16 muls in 2x tensor_tensor mode ~1224ns/2048 cols), combines, and
    does the strided r-reduce.  The cross-btile ops (V, Sn) and deferred
    finalization are placed so they fill DVE's idle window at btile start.
    bf16 (not fp16) because DVE's fast 16-bit uop programs are bf16-only
    (fp16 tensor_tensor measured ~2x SLOWER than fp32).
  * Input DMA split across two hardware DGE rings (sync + gpsimd queues) in
    consumption order so the first matmul starts ~2.5us in, not ~12us.
"""

import numpy as np

B, F, D, R, U = 1024, 64, 4, 32, 128
NCORES = 8
BSH, USH = 2, 4                 # batch shards x unit shards
BLOC = B // BSH                 # 512 batch rows per core
NBT = BLOC // 128               # 4 batch tiles of 128
ULOC = U // USH                 # 32 units per core
RUL = R * ULOC                  # 1024 columns (u-major: col = u*32 + r)
NQ = 4                          # quads (features 0..15)
NT = 16                         # triples (features 16..63)
NPAIR = NT // 2                 # triple pairs (kt slots 0..7)
NSLOT = NPAIR + 2 * NQ          # kt slots: 8 triple-pairs + 2 per quad
NW = 10                         # wide pairs per btile (8 triple + 2 quad)

FUSED_W = (0, 1, 5)             # DVE narrow fp32 fused chain pairs
ACT_W = (2, 3, 4, 6, 7, 8, 9)   # Act wide drains -> bf16 (B1..B7)

_cached = {}


def _build_nc():
    import concourse.bass as bass
    import concourse.mybir as mybir
    import concourse.tile as tile
    from concourse import bacc

    fp32 = mybir.dt.float32
    fp16 = mybir.dt.float16
    bf16 = mybir.dt.bfloat16
    nc = bacc.Bacc("TRN2", target_bir_lowering=False, debug=False)

    xt_d = nc.dram_tensor("xt", [NBT, 128, NSLOT * 128], fp16, kind="ExternalInput").ap()
    kt_d = nc.dram_tensor("kt", [NSLOT, 128, RUL], fp16, kind="ExternalInput").ap()
    out_d = nc.dram_tensor("out", [BLOC, ULOC], fp32, kind="ExternalOutput").ap()

    W2 = 2 * RUL                # 2048 wide columns

    with tile.TileContext(nc) as tc:
        with (
            tc.tile_pool(name="kt", bufs=1) as ktpool,
            tc.tile_pool(name="xt", bufs=1) as xtpool,
            tc.tile_pool(name="fb", bufs=7) as fbpool,
            tc.tile_pool(name="ff", bufs=2) as ffpool,
            tc.tile_pool(name="cc", bufs=2) as ccpool,
            tc.tile_pool(name="tt", bufs=2) as ttpool,
            tc.tile_pool(name="sn", bufs=2) as snpool,
            tc.tile_pool(name="oo", bufs=2) as oopool,
            tc.tile_pool(name="out", bufs=2) as outpool,
            tc.tile_pool(name="ps", bufs=2, space="PSUM") as pspool,
        ):
            # --- input DMA on two rings, consumption-ordered ---
            xts = [
                xtpool.tile([128, NSLOT * 128], fp16, tag=f"xt{t}", name=f"xt{t}")
                for t in range(NBT)
            ]
            kts = [
                ktpool.tile([128, RUL], fp16, tag=f"kt{s}", name=f"kt{s}")
                for s in range(NSLOT)
            ]
            # ring A (sync): xt0, even kt slots, xt1-3; ring B (gpsimd): odd
            nc.gpsimd.dma_start(kts[0][:], kt_d[0])
            nc.sync.dma_start(xts[0][:], xt_d[0])
            nc.gpsimd.dma_start(kts[1][:], kt_d[1])
            for s in range(2, NSLOT, 2):
                nc.sync.dma_start(kts[s][:], kt_d[s])
                nc.gpsimd.dma_start(kts[s + 1][:], kt_d[s + 1])
            for t in range(1, NBT):
                nc.sync.dma_start(xts[t][:], xt_d[t])

            pending = []  # deferred finalization thunks (one btile late)

            def xsl(s):
                return slice(s * 128, (s + 1) * 128)

            for t in range(NBT):
                xt = xts[t]

                # narrow fp32 fused chain (ping-pong, out never aliases in)
                Gb = [
                    ffpool.tile([128, RUL], fp32, tag=f"G{i}", name=f"G{i}")
                    for i in range(2)
                ]
                Ub = [
                    ccpool.tile([128, W2], bf16, tag=f"U{i}", name=f"U{i}")
                    for i in range(2)
                ]
                Tb = [
                    ttpool.tile([128, W2], bf16, tag=f"T{i}", name=f"T{i}")
                    for i in range(2)
                ]
                Vw = ttpool.tile([128, W2], bf16, tag="V", name="V")
                Sn = snpool.tile([128, RUL], bf16, tag="Sn", name="Sn")

                bbufs = {}
                nact = 0
                ng = 0

                def emit_pair_matmuls(w, ps):
                    if w < NPAIR:  # triple pair: kt slot w, row groups h0/h64
                        for c in range(2):
                            for s in range(2):
                                rows = slice(64 * s, 64 * s + 64)
                                dcs = slice(
                                    1024 * s + 512 * c, 1024 * s + 512 * c + 512
                                )
                                scs = slice(512 * c, 512 * c + 512)
                                nc.tensor.matmul(
                                    ps[:, dcs],
                                    xt[rows, xsl(w)],
                                    kts[w][rows, scs],
                                    start=True,
                                    stop=True,
                                    tile_position=(64 * s, 0),
                                )
                    else:  # quad pair: quads 2*(w-8), 2*(w-8)+1
                        for j in range(2):
                            q = 2 * (w - NPAIR) + j
                            for h in range(2):
                                slot = NPAIR + 2 * q + h
                                for c in range(2):
                                    dcs = slice(
                                        1024 * j + 512 * c, 1024 * j + 512 * c + 512
                                    )
                                    scs = slice(512 * c, 512 * c + 512)
                                    nc.tensor.matmul(
                                        ps[:, dcs],
                                        xt[:, xsl(slot)],
                                        kts[slot][:, scs],
                                        start=(h == 0),
                                        stop=(h == 1),
                                    )

                for w in range(NW):
                    ps = pspool.tile([128, W2], fp32, tag="ps", name="ps")
                    emit_pair_matmuls(w, ps)

                    if w in FUSED_W:
                        # two narrow single-PSUM ops consume the pair
                        for h in range(2):
                            half = ps[:, h * RUL : (h + 1) * RUL]
                            if ng == 0:
                                nc.vector.tensor_copy(Gb[0][:], half)
                            else:
                                nc.vector.tensor_mul(
                                    Gb[ng % 2][:], Gb[(ng - 1) % 2][:], half
                                )
                            ng += 1
                    else:
                        fb = fbpool.tile([128, W2], bf16, tag="fb", name="fb")
                        nc.scalar.copy(fb[:], ps[:])
                        bbufs[w] = fb
                        nact += 1
                        # Pool: U-chain over B1..B3 (pairs W2,W3,W4)
                        if nact == 2:
                            nc.gpsimd.tensor_mul(
                                Ub[0][:], bbufs[2][:], bbufs[3][:]
                            )
                        elif nact == 3:
                            nc.gpsimd.tensor_mul(Ub[1][:], Ub[0][:], fb[:])
                        # DVE: T-chain over B4..B7 (pairs W6,W7,W8,W9)
                        elif nact == 5:
                            nc.vector.tensor_mul(
                                Tb[0][:], bbufs[6][:], bbufs[7][:]
                            )
                        elif nact == 6:
                            nc.vector.tensor_mul(Tb[1][:], Tb[0][:], fb[:])
                        elif nact == 7:
                            nc.vector.tensor_mul(Tb[0][:], Tb[1][:], fb[:])

                    if w == 2 and pending:
                        # previous btile's finalization (O on Pool, reduce
                        # on DVE) lands in the W2..W4 Act-drain window
                        pending.pop(0)()

                # cross-btile ops: ride DVE's idle ramp of the next btile
                nc.vector.tensor_mul(Vw[:], Tb[0][:], Ub[1][:])
                nc.vector.tensor_mul(Sn[:], Vw[:, 0:RUL], Vw[:, RUL:W2])

                def finalize(t=t, G=Gb[1], Sn=Sn):
                    O = oopool.tile([128, RUL], fp32, tag="O", name="O")
                    osum = outpool.tile([128, ULOC], fp32, tag="osum", name="osum")
                    nc.gpsimd.tensor_mul(O[:], G[:], Sn[:])
                    nc.vector.tensor_reduce(
                        osum[:],
                        O[:].rearrange("p (u r) -> p u r", r=R),
                        axis=mybir.AxisListType.X,
                        op=mybir.AluOpType.add,
                    )
                    nc.sync.dma_start(out_d[t * 128 : (t + 1) * 128, :], osum[:])

                pending.append(finalize)

            for fin in pending:
                fin()

    nc.compile()
    return nc


def _host_prep(X, K):
    """Repack inputs into per-core fp16 stationary/moving operands.

    Quad q covers features 4q..4q+3 as two K=128 PSUM-accumulated passes
    (row = ((d0*4+d1)*4+d2)*2 + l, l indexing half of the 4th feature's
    d range).  Triples cover features 48+3j..50+3j (row = d0*16+d1*4+d2),
    two per kt slot (rows 0:64 / 64:128) for row-tiled matmul pairs;
    feature 63 rides in the third pair's B half (rows 64:68).  Columns are
    u-major (col = u*32 + r).
    """
    f16 = np.float16
    FT = 4 * NQ                      # first triple feature
    kt_cores, xt_cores = [], []
    for bi in range(BSH):
        Xc = X[bi * BLOC : (bi + 1) * BLOC]                    # [512, 64, 4]
        for uj in range(USH):
            Ku = K[:, :, :, uj * ULOC : (uj + 1) * ULOC]       # [4,32,64,32]
            Kf = np.ascontiguousarray(
                Ku.transpose(2, 0, 3, 1).reshape(F, D, RUL)
            )                                                   # [f, d, col]
            kt = np.zeros((NSLOT, 128, RUL), dtype=f16)
            xt = np.zeros((NBT, 128, NSLOT * 128), dtype=f16)

            def put_x(slot, rows, arr):  # arr [BLOC, nrows]
                for t in range(NBT):
                    xt[t, rows, slot * 128 : (slot + 1) * 128] = arr[
                        t * 128 : (t + 1) * 128
                    ].T

            # triple pairs in slots 0..NPAIR-1
            for p in range(NPAIR):
                for s in range(2):
                    j = 2 * p + s
                    rows = slice(64 * s, 64 * s + 64)
                    f0 = FT + 3 * j
                    K3 = (
                        Kf[f0][:, None, None, :]
                        * Kf[f0 + 1][None, :, None, :]
                        * Kf[f0 + 2][None, None, :, :]
                    ).reshape(64, RUL)
                    X3 = (
                        Xc[:, f0, :, None, None]
                        * Xc[:, f0 + 1, None, :, None]
                        * Xc[:, f0 + 2, None, None, :]
                    ).reshape(BLOC, 64)
                    kt[p, rows] = K3
                    put_x(p, rows, X3)
            # quads in slots NPAIR + 2q + h
            for q in range(NQ):
                f0 = 4 * q
                K012 = (
                    Kf[f0][:, None, None, :]
                    * Kf[f0 + 1][None, :, None, :]
                    * Kf[f0 + 2][None, None, :, :]
                ).reshape(64, RUL)
                X012 = (
                    Xc[:, f0, :, None, None]
                    * Xc[:, f0 + 1, None, :, None]
                    * Xc[:, f0 + 2, None, None, :]
                ).reshape(BLOC, 64)
                for h in range(2):
                    slot = NPAIR + 2 * q + h
                    kt[slot] = (
                        K012[:, None, :] * Kf[f0 + 3][2 * h : 2 * h + 2][None, :, :]
                    ).reshape(128, RUL)
                    X4h = (
                        X012[:, :, None]
                        * Xc[:, f0 + 3, 2 * h : 2 * h + 2][:, None, :]
                    ).reshape(BLOC, 128)
                    put_x(slot, slice(0, 128), X4h)
            kt_cores.append(np.ascontiguousarray(kt))
            xt_cores.append(np.ascontiguousarray(xt))
    return [{"xt": xt_cores[c], "kt": kt_cores[c]} for c in range(NCORES)]


def kernel(**inputs):
    from concourse.bass_utils import run_bass_kernel_spmd

    X = np.asarray(inputs["X"], dtype=np.float32)
    K = np.asarray(inputs["kernel"], dtype=np.float32)
    assert X.shape == (B, F, D) and K.shape == (D, R, F, U)

    if "nc" not in _cached:
        _cached["nc"] = _build_nc()
    nc = _cached["nc"]

    in_maps = _host_prep(X, K)
    res = run_bass_kernel_spmd(nc, in_maps, core_ids=list(range(NCORES)))
    out = np.zeros((B, U), dtype=np.float32)
    for c in range(NCORES):
        bi, uj = divmod(c, USH)
        out[bi * BLOC : (bi + 1) * BLOC, uj * ULOC : (uj + 1) * ULOC] = res.results[
            c
        ]["out"]
    return out


# revision 9
# speedup vs baseline: 1.0307x; 1.0307x over previous
"""Trainium2 Bass kernel for nn_CP_Based (CP-decomposition interaction layer).

Math (full problem):
    t[b,f,r,u] = sum_d X[b,f,d] * K[d,r,f,u]      (B=1024, F=64, D=4, R=32, U=128)
    had[b,r,u] = prod_f t[b,f,r,u]
    out[b,u]   = sum_r had[b,r,u]

Strategy (v6):
  * Shard batch x units across 8 cores as (2 batch halves) x (4 unit
    quarters): per core B_loc=512 (4 partition tiles of 128) and RU_loc =
    32r x 32u = 1024 columns (u-major, r contiguous for the final reduce).
  * Host-side feature grouping: 16 triples (K=64, row-group pairs sharing a
    kt slot) + 4 quads (K=256 as 2 PSUM-accumulated K=128 passes) = 20
    factor tiles [128,1024] per batch tile.  Matmul inputs fp16.
  * PSUM ring: narrow [128,1024] tiles, bufs=4 (8 banks).  ALL consumers are
    narrow single-tile ops so every bank releases independently at ~PE pace
    (wide 2-slot rings serialize mm->consume and pace the whole btile).
  * Consumers: 7 tiles (incl. ALL 4 quads - numerically the touchiest) fold
    into a narrow fp32 fused chain G on DVE (one PSUM operand per op, no
    16-bit quantization).  13 tiles drain via Act ACTIVATE into bf16, packed
    pairwise into [128,2048] wide buffers so the product tree runs as wide
    bf16 muls in the DVE 2x tensor_tensor mode (~1.22us/2048 cols).  bf16,
    not fp16: DVE's fast 16-bit uop programs are bf16-only.
  * Tree: DVE chains 3 wide bufs, Pool chains the other 3 (2 wide muls),
    DVE combines + folds + handles the leftover narrow tile; the final
    merge with G runs on Pool and the strided r-reduce on DVE.  Cross-btile
    tail ops ride DVE's idle ramp of the next btile; finalization is
    deferred one btile (emitted mid-btile so queues never head-of-line
    block).
  * Input DMA split across two DGE rings (sync + gpsimd) in consumption
    order so the first matmul starts ~2.5us in.
"""

import numpy as np

B, F, D, R, U = 1024, 64, 4, 32, 128
NCORES = 8
BSH, USH = 2, 4                 # batch shards x unit shards
BLOC = B // BSH                 # 512 batch rows per core
NBT = BLOC // 128               # 4 batch tiles of 128
ULOC = U // USH                 # 32 units per core
RUL = R * ULOC                  # 1024 columns (u-major: col = u*32 + r)
NQ = 4                          # quads (features 0..15)
NT = 16                         # triples (features 16..63)
NTILE = NT + NQ                 # 20 factor tiles per batch tile
NPAIR = NT // 2                 # triple pairs (kt slots 0..7)
NSLOT = NPAIR + 2 * NQ          # kt slots: 8 triple-pairs + 2 per quad

FUSED = (2, 5, 8, 16, 17, 18, 19)   # DVE fp32 fused chain (all quads anchored)
ACTS = tuple(i for i in range(NTILE) if i not in FUSED)  # 13 Act drains
# pack consecutive Act tiles into halves of wide bf16 buffers
APAIR = tuple((ACTS[2 * k], ACTS[2 * k + 1]) for k in range(len(ACTS) // 2))
ALONE = ACTS[-1]                # leftover narrow tile (15)

_cached = {}


def _build_nc():
    import concourse.bass as bass
    import concourse.mybir as mybir
    import concourse.tile as tile
    from concourse import bacc

    fp32 = mybir.dt.float32
    fp16 = mybir.dt.float16
    bf16 = mybir.dt.bfloat16
    nc = bacc.Bacc("TRN2", target_bir_lowering=False, debug=False)

    xt_d = nc.dram_tensor("xt", [NBT, 128, NSLOT * 128], fp16, kind="ExternalInput").ap()
    kt_d = nc.dram_tensor("kt", [NSLOT, 128, RUL], fp16, kind="ExternalInput").ap()
    out_d = nc.dram_tensor("out", [BLOC, ULOC], fp32, kind="ExternalOutput").ap()

    W2 = 2 * RUL

    # map act tile -> (pair index, half) or None for the leftover
    half_of = {}
    for k, (i, j) in enumerate(APAIR):
        half_of[i] = (k, 0)
        half_of[j] = (k, 1)

    with tile.TileContext(nc) as tc:
        with (
            tc.tile_pool(name="kt", bufs=1) as ktpool,
            tc.tile_pool(name="xt", bufs=1) as xtpool,
            tc.tile_pool(name="fb", bufs=12) as fbpool,
            tc.tile_pool(name="fn", bufs=2) as fnpool,
            tc.tile_pool(name="gg", bufs=2) as ggpool,
            tc.tile_pool(name="tt", bufs=2) as ttpool,
            tc.tile_pool(name="uu", bufs=2) as uupool,
            tc.tile_pool(name="vv", bufs=2) as vvpool,
            tc.tile_pool(name="oo", bufs=2) as oopool,
            tc.tile_pool(name="out", bufs=2) as outpool,
            tc.tile_pool(name="ps", bufs=4, space="PSUM") as pspool,
        ):
            xts = [
                xtpool.tile([128, NSLOT * 128], fp16, tag=f"xt{t}", name=f"xt{t}")
                for t in range(NBT)
            ]
            kts = [
                ktpool.tile([128, RUL], fp16, tag=f"kt{s}", name=f"kt{s}")
                for s in range(NSLOT)
            ]
            nc.gpsimd.dma_start(kts[0][:], kt_d[0])
            nc.sync.dma_start(xts[0][:], xt_d[0])
            nc.gpsimd.dma_start(kts[1][:], kt_d[1])
            for s in range(2, NSLOT, 2):
                nc.sync.dma_start(kts[s][:], kt_d[s])
                nc.gpsimd.dma_start(kts[s + 1][:], kt_d[s + 1])
            for t in range(1, NBT):
                nc.sync.dma_start(xts[t][:], xt_d[t])

            pending = []

            def xsl(s):
                return slice(s * 128, (s + 1) * 128)

            for t in range(NBT):
                xt = xts[t]

                Gb = [
                    ggpool.tile([128, RUL], fp32, tag=f"G{i}", name=f"G{i}")
                    for i in range(2)
                ]
                Tb = [
                    ttpool.tile([128, W2], bf16, tag=f"T{i}", name=f"T{i}")
                    for i in range(2)
                ]
                Ub = [
                    uupool.tile([128, W2], bf16, tag=f"U{i}", name=f"U{i}")
                    for i in range(2)
                ]
                Vw = vvpool.tile([128, W2], bf16, tag="V", name="V")
                Sn = vvpool.tile([128, RUL], bf16, tag="Sn", name="Sn")
                Sn2 = vvpool.tile([128, RUL], bf16, tag="Sn2", name="Sn2")
                fbs = [
                    fbpool.tile([128, W2], bf16, tag="fb", name="fb")
                    for _ in range(len(APAIR))
                ]
                fnn = fnpool.tile([128, RUL], bf16, tag="fn", name="fn")

                ng = 0
                for i in range(NTILE):
                    ps = pspool.tile([128, RUL], fp32, tag="ps", name="ps")
                    if i < NT:  # triple half: pair p = i//2, sub s = i%2
                        p, s = divmod(i, 2)
                        rows = slice(64 * s, 64 * s + 64)
                        for h in range(2):
                            cs = slice(512 * h, 512 * h + 512)
                            nc.tensor.matmul(
                                ps[:, cs],
                                xt[rows, xsl(p)],
                                kts[p][rows, cs],
                                start=True,
                                stop=True,
                                tile_position=(64 * s, 0),
                            )
                    else:  # quad: 2 psum-accumulated K=128 passes
                        q = i - NT
                        for h in range(2):
                            slot = NPAIR + 2 * q + h
                            for c in range(2):
                                cs = slice(512 * c, 512 * c + 512)
                                nc.tensor.matmul(
                                    ps[:, cs],
                                    xt[:, xsl(slot)],
                                    kts[slot][:, cs],
                                    start=(h == 0),
                                    stop=(h == 1),
                                )

                    if i in FUSED:
                        if ng == 0:
                            nc.vector.tensor_copy(Gb[0][:], ps[:])
                        else:
                            nc.vector.tensor_mul(
                                Gb[ng % 2][:], Gb[(ng - 1) % 2][:], ps[:]
                            )
                        ng += 1
                    elif i == ALONE:
                        nc.scalar.copy(fnn[:], ps[:])
                    else:
                        k, h = half_of[i]
                        nc.scalar.copy(fbs[k][:, h * RUL : (h + 1) * RUL], ps[:])
                        if i == APAIR[1][1]:       # B0,B1 drained
                            nc.vector.tensor_mul(Tb[0][:], fbs[0][:], fbs[1][:])
                        elif i == APAIR[2][1]:     # B2 drained
                            nc.vector.tensor_mul(Tb[1][:], Tb[0][:], fbs[2][:])
                        elif i == APAIR[4][1]:     # B3,B4 drained
                            nc.gpsimd.tensor_mul(Ub[0][:], fbs[3][:], fbs[4][:])
                        elif i == APAIR[5][1]:     # B5 drained
                            nc.gpsimd.tensor_mul(Ub[1][:], Ub[0][:], fbs[5][:])

                    if i == 6 and pending:
                        pending.pop(0)()

                # cross-btile tail: rides the next btile's DVE ramp window
                nc.vector.tensor_mul(Vw[:], Tb[1][:], Ub[1][:])
                nc.vector.tensor_mul(Sn[:], Vw[:, 0:RUL], Vw[:, RUL:W2])
                nc.vector.tensor_mul(Sn2[:], Sn[:], fnn[:])

                def finalize(t=t, G=Gb[(len(FUSED) - 1) % 2], Sn2=Sn2):
                    O = oopool.tile([128, RUL], fp32, tag="O", name="O")
                    osum = outpool.tile([128, ULOC], fp32, tag="osum", name="osum")
                    nc.gpsimd.tensor_mul(O[:], G[:], Sn2[:])
                    nc.vector.tensor_reduce(
                        osum[:],
                        O[:].rearrange("p (u r) -> p u r", r=R),
                        axis=mybir.AxisListType.X,
                        op=mybir.AluOpType.add,
                    )
                    nc.sync.dma_start(out_d[t * 128 : (t + 1) * 128, :], osum[:])

                pending.append(finalize)

            for fin in pending:
                fin()

    nc.compile()
    return nc


def _host_prep(X, K):
    """Repack inputs into per-core fp16 stationary/moving operands.

    Quad q covers features 4q..4q+3 as two K=128 PSUM-accumulated passes
    (row = ((d0*4+d1)*4+d2)*2 + l, l indexing half of the 4th feature's
    d range).  Triples cover features 48+3j..50+3j (row = d0*16+d1*4+d2),
    two per kt slot (rows 0:64 / 64:128) for row-tiled matmul pairs;
    feature 63 rides in the third pair's B half (rows 64:68).  Columns are
    u-major (col = u*32 + r).
    """
    f16 = np.float16
    FT = 4 * NQ                      # first triple feature
    kt_cores, xt_cores = [], []
    for bi in range(BSH):
        Xc = X[bi * BLOC : (bi + 1) * BLOC]                    # [512, 64, 4]
        for uj in range(USH):
            Ku = K[:, :, :, uj * ULOC : (uj + 1) * ULOC]       # [4,32,64,32]
            Kf = np.ascontiguousarray(
                Ku.transpose(2, 0, 3, 1).reshape(F, D, RUL)
            )                                                   # [f, d, col]
            kt = np.zeros((NSLOT, 128, RUL), dtype=f16)
            xt = np.zeros((NBT, 128, NSLOT * 128), dtype=f16)

            def put_x(slot, rows, arr):  # arr [BLOC, nrows]
                for t in range(NBT):
                    xt[t, rows, slot * 128 : (slot + 1) * 128] = arr[
                        t * 128 : (t + 1) * 128
                    ].T

            # triple pairs in slots 0..NPAIR-1
            for p in range(NPAIR):
                for s in range(2):
                    j = 2 * p + s
                    rows = slice(64 * s, 64 * s + 64)
                    f0 = FT + 3 * j
                    K3 = (
                        Kf[f0][:, None, None, :]
                        * Kf[f0 + 1][None, :, None, :]
                        * Kf[f0 + 2][None, None, :, :]
                    ).reshape(64, RUL)
                    X3 = (
                        Xc[:, f0, :, None, None]
                        * Xc[:, f0 + 1, None, :, None]
                        * Xc[:, f0 + 2, None, None, :]
                    ).reshape(BLOC, 64)
                    kt[p, rows] = K3
                    put_x(p, rows, X3)
            # quads in slots NPAIR + 2q + h
            for q in range(NQ):
                f0 = 4 * q
                K012 = (
                    Kf[f0][:, None, None, :]
                    * Kf[f0 + 1][None, :, None, :]
                    * Kf[f0 + 2][None, None, :, :]
                ).reshape(64, RUL)
                X012 = (
                    Xc[:, f0, :, None, None]
                    * Xc[:, f0 + 1, None, :, None]
                    * Xc[:, f0 + 2, None, None, :]
                ).reshape(BLOC, 64)
                for h in range(2):
                    slot = NPAIR + 2 * q + h
                    kt[slot] = (
                        K012[:, None, :] * Kf[f0 + 3][2 * h : 2 * h + 2][None, :, :]
                    ).reshape(128, RUL)
                    X4h = (
                        X012[:, :, None]
                        * Xc[:, f0 + 3, 2 * h : 2 * h + 2][:, None, :]
                    ).reshape(BLOC, 128)
                    put_x(slot, slice(0, 128), X4h)
            kt_cores.append(np.ascontiguousarray(kt))
            xt_cores.append(np.ascontiguousarray(xt))
    return [{"xt": xt_cores[c], "kt": kt_cores[c]} for c in range(NCORES)]


def kernel(**inputs):
    from concourse.bass_utils import run_bass_kernel_spmd

    X = np.asarray(inputs["X"], dtype=np.float32)
    K = np.asarray(inputs["kernel"], dtype=np.float32)
    assert X.shape == (B, F, D) and K.shape == (D, R, F, U)

    if "nc" not in _cached:
        _cached["nc"] = _build_nc()
    nc = _cached["nc"]

    in_maps = _host_prep(X, K)
    res = run_bass_kernel_spmd(nc, in_maps, core_ids=list(range(NCORES)))
    out = np.zeros((B, U), dtype=np.float32)
    for c in range(NCORES):
        bi, uj = divmod(c, USH)
        out[bi * BLOC : (bi + 1) * BLOC, uj * ULOC : (uj + 1) * ULOC] = res.results[
            c
        ]["out"]
    return out


# revision 15
# speedup vs baseline: 1.0505x; 1.0192x over previous
"""Trainium2 Bass kernel for nn_CP_Based (CP-decomposition interaction layer).

Math (full problem):
    t[b,f,r,u] = sum_d X[b,f,d] * K[d,r,f,u]      (B=1024, F=64, D=4, R=32, U=128)
    had[b,r,u] = prod_f t[b,f,r,u]
    out[b,u]   = sum_r had[b,r,u]

Strategy (v6):
  * Shard batch x units across 8 cores as (2 batch halves) x (4 unit
    quarters): per core B_loc=512 (4 partition tiles of 128) and RU_loc =
    32r x 32u = 1024 columns (u-major, r contiguous for the final reduce).
  * Host-side feature grouping: 16 triples (K=64, row-group pairs sharing a
    kt slot) + 4 quads (K=256 as 2 PSUM-accumulated K=128 passes) = 20
    factor tiles [128,1024] per batch tile.  Matmul inputs fp16.
  * PSUM ring: narrow [128,1024] tiles, bufs=4 (8 banks).  ALL consumers are
    narrow single-tile ops so every bank releases independently at ~PE pace
    (wide 2-slot rings serialize mm->consume and pace the whole btile).
  * Consumers: 7 tiles (incl. ALL 4 quads - numerically the touchiest) fold
    into a narrow fp32 fused chain G on DVE (one PSUM operand per op, no
    16-bit quantization).  13 tiles drain via Act ACTIVATE into bf16, packed
    pairwise into [128,2048] wide buffers so the product tree runs as wide
    bf16 muls in the DVE 2x tensor_tensor mode (~1.22us/2048 cols).  bf16,
    not fp16: DVE's fast 16-bit uop programs are bf16-only.
  * Tree: DVE chains 3 wide bufs, Pool chains the other 3 (2 wide muls),
    DVE combines + folds + handles the leftover narrow tile; the final
    merge with G runs on Pool and the strided r-reduce on DVE.  Cross-btile
    tail ops ride DVE's idle ramp of the next btile; finalization is
    deferred one btile (emitted mid-btile so queues never head-of-line
    block).
  * Input DMA split across two DGE rings (sync + gpsimd) in consumption
    order so the first matmul starts ~2.5us in.
"""

import numpy as np

B, F, D, R, U = 1024, 64, 4, 32, 128
NCORES = 8
BSH, USH = 2, 4                 # batch shards x unit shards
BLOC = B // BSH                 # 512 batch rows per core
NBT = BLOC // 128               # 4 batch tiles of 128
ULOC = U // USH                 # 32 units per core
RUL = R * ULOC                  # 1024 columns (u-major: col = u*32 + r)
NQ = 4                          # quads (features 0..15)
NT = 16                         # triples (features 16..63)
NTILE = NT + NQ                 # 20 factor tiles per batch tile
NPAIR = NT // 2                 # triple pairs (kt slots 0..7)
NSLOT = NPAIR + 2 * NQ          # kt slots: 8 triple-pairs + 2 per quad

FUSED = (2, 5, 8, 16, 17, 18, 19)   # DVE fp32 fused chain (all quads anchored)
ACTS = tuple(i for i in range(NTILE) if i not in FUSED)  # 13 Act drains
# pack consecutive Act tiles into halves of wide bf16 buffers
APAIR = tuple((ACTS[2 * k], ACTS[2 * k + 1]) for k in range(len(ACTS) // 2))
ALONE = ACTS[-1]                # leftover narrow tile (15)

_cached = {}


def _build_nc():
    import concourse.bass as bass
    import concourse.mybir as mybir
    import concourse.tile as tile
    from concourse import bacc

    fp32 = mybir.dt.float32
    fp16 = mybir.dt.float16
    bf16 = mybir.dt.bfloat16
    nc = bacc.Bacc("TRN2", target_bir_lowering=False, debug=False)

    xt_d = nc.dram_tensor("xt", [NBT, 128, NSLOT * 128], fp16, kind="ExternalInput").ap()
    kt_d = nc.dram_tensor("kt", [NSLOT, 128, RUL], fp16, kind="ExternalInput").ap()
    out_d = nc.dram_tensor("out", [BLOC, ULOC], fp32, kind="ExternalOutput").ap()

    W2 = 2 * RUL

    # map act tile -> (pair index, half) or None for the leftover
    half_of = {}
    for k, (i, j) in enumerate(APAIR):
        half_of[i] = (k, 0)
        half_of[j] = (k, 1)

    with tile.TileContext(nc) as tc:
        with (
            tc.tile_pool(name="kt", bufs=1) as ktpool,
            tc.tile_pool(name="xt", bufs=1) as xtpool,
            tc.tile_pool(name="fb", bufs=12) as fbpool,
            tc.tile_pool(name="fn", bufs=2) as fnpool,
            tc.tile_pool(name="gg", bufs=2) as ggpool,
            tc.tile_pool(name="tt", bufs=2) as ttpool,
            tc.tile_pool(name="uu", bufs=2) as uupool,
            tc.tile_pool(name="vv", bufs=2) as vvpool,
            tc.tile_pool(name="oo", bufs=2) as oopool,
            tc.tile_pool(name="out", bufs=2) as outpool,
            tc.tile_pool(name="ps", bufs=4, space="PSUM") as pspool,
        ):
            xts = [
                xtpool.tile([128, NSLOT * 128], fp16, tag=f"xt{t}", name=f"xt{t}")
                for t in range(NBT)
            ]
            kts = [
                ktpool.tile([128, RUL], fp16, tag=f"kt{s}", name=f"kt{s}")
                for s in range(NSLOT)
            ]
            # three DGE rings: sync + scalar (HW) and gpsimd (SW).  First
            # matmul needs xt0 cols 0:128 + kt0; split xt0 into consumption-
            # ordered column chunks and fan the urgent transfers out.
            nc.scalar.dma_start(kts[0][:], kt_d[0])
            nc.sync.dma_start(xts[0][:], xt_d[0])
            nc.gpsimd.dma_start(kts[1][:], kt_d[1])
            nc.gpsimd.dma_start(kts[2][:], kt_d[2])
            nc.gpsimd.dma_start(kts[3][:], kt_d[3])
            for s in range(4, NSLOT, 2):
                nc.sync.dma_start(kts[s][:], kt_d[s])
                nc.gpsimd.dma_start(kts[s + 1][:], kt_d[s + 1])
            for t in range(1, NBT):
                nc.sync.dma_start(xts[t][:], xt_d[t])

            pending = []

            def xsl(s):
                return slice(s * 128, (s + 1) * 128)

            for t in range(NBT):
                xt = xts[t]

                Gb = [
                    ggpool.tile([128, RUL], fp32, tag=f"G{i}", name=f"G{i}")
                    for i in range(2)
                ]
                Tb = [
                    ttpool.tile([128, W2], bf16, tag=f"T{i}", name=f"T{i}")
                    for i in range(2)
                ]
                Ub = [
                    uupool.tile([128, W2], bf16, tag=f"U{i}", name=f"U{i}")
                    for i in range(2)
                ]
                Vw = vvpool.tile([128, W2], bf16, tag="V", name="V")
                Sn = vvpool.tile([128, RUL], bf16, tag="Sn", name="Sn")
                Sn2 = vvpool.tile([128, RUL], bf16, tag="Sn2", name="Sn2")
                fbs = [
                    fbpool.tile([128, W2], bf16, tag="fb", name="fb")
                    for _ in range(len(APAIR))
                ]
                fnn = fnpool.tile([128, RUL], bf16, tag="fn", name="fn")

                ng = 0
                for i in range(NTILE):
                    ps = pspool.tile([128, RUL], fp32, tag="ps", name="ps")
                    if i < NT:  # triple half: pair p = i//2, sub s = i%2
                        p, s = divmod(i, 2)
                        rows = slice(64 * s, 64 * s + 64)
                        for h in range(2):
                            cs = slice(512 * h, 512 * h + 512)
                            nc.tensor.matmul(
                                ps[:, cs],
                                xt[rows, xsl(p)],
                                kts[p][rows, cs],
                                start=True,
                                stop=True,
                                tile_position=(64 * s, 0),
                            )
                    else:  # quad: 2 psum-accumulated K=128 passes
                        q = i - NT
                        for h in range(2):
                            slot = NPAIR + 2 * q + h
                            for c in range(2):
                                cs = slice(512 * c, 512 * c + 512)
                                nc.tensor.matmul(
                                    ps[:, cs],
                                    xt[:, xsl(slot)],
                                    kts[slot][:, cs],
                                    start=(h == 0),
                                    stop=(h == 1),
                                )

                    if i in FUSED:
                        if ng == 0:
                            nc.vector.tensor_copy(Gb[0][:], ps[:])
                        else:
                            nc.vector.tensor_mul(
                                Gb[ng % 2][:], Gb[(ng - 1) % 2][:], ps[:]
                            )
                        ng += 1
                    elif i == ALONE:
                        nc.scalar.copy(fnn[:], ps[:])
                        if t == NBT - 1:
                            nc.vector.tensor_mul(Sn2[:], Sn[:], fnn[:])
                    else:
                        k, h = half_of[i]
                        nc.scalar.copy(fbs[k][:, h * RUL : (h + 1) * RUL], ps[:])
                        # Pool (slow, 4.1us/wide) gets the EARLY buffers so it
                        # finishes within the btile; DVE (fast) gets the LATE
                        # ones so the cross-btile tail is short.
                        if i == APAIR[1][1]:       # B0,B1 drained
                            nc.gpsimd.tensor_mul(Ub[0][:], fbs[0][:], fbs[1][:])
                        elif i == APAIR[2][1]:     # B2 drained
                            nc.gpsimd.tensor_mul(Ub[1][:], Ub[0][:], fbs[2][:])
                        elif i == APAIR[4][1]:     # B3,B4 drained
                            nc.vector.tensor_mul(Tb[0][:], fbs[3][:], fbs[4][:])
                        elif i == APAIR[5][1]:     # B5 drained
                            nc.vector.tensor_mul(Tb[1][:], Tb[0][:], fbs[5][:])

                    if i == 2 and pending:
                        pending.pop(0)()

                    # last btile: emit the tail right after T1 so only the
                    # G-chain end + finalize remain after the final matmul
                    # (Sn2 is emitted at the ALONE drain, which comes later)
                    if t == NBT - 1 and i == APAIR[5][1]:
                        nc.vector.tensor_mul(Vw[:], Tb[1][:], Ub[1][:])
                        nc.vector.tensor_mul(Sn[:], Vw[:, 0:RUL], Vw[:, RUL:W2])

                if t < NBT - 1:
                    # cross-btile tail: rides the next btile's DVE ramp window
                    nc.vector.tensor_mul(Vw[:], Tb[1][:], Ub[1][:])
                    nc.vector.tensor_mul(Sn[:], Vw[:, 0:RUL], Vw[:, RUL:W2])
                    nc.vector.tensor_mul(Sn2[:], Sn[:], fnn[:])

                def finalize(t=t, G=Gb[(len(FUSED) - 1) % 2], Sn2=Sn2):
                    O = oopool.tile([128, RUL], fp32, tag="O", name="O")
                    osum = outpool.tile([128, ULOC], fp32, tag="osum", name="osum")
                    nc.gpsimd.tensor_mul(O[:], G[:], Sn2[:])
                    nc.vector.tensor_reduce(
                        osum[:],
                        O[:].rearrange("p (u r) -> p u r", r=R),
                        axis=mybir.AxisListType.X,
                        op=mybir.AluOpType.add,
                    )
                    nc.sync.dma_start(out_d[t * 128 : (t + 1) * 128, :], osum[:])

                pending.append(finalize)

            for fin in pending:
                fin()

    nc.compile()
    return nc


def _host_prep(X, K):
    """Repack inputs into per-core fp16 stationary/moving operands.

    Quad q covers features 4q..4q+3 as two K=128 PSUM-accumulated passes
    (row = ((d0*4+d1)*4+d2)*2 + l, l indexing half of the 4th feature's
    d range).  Triples cover features 48+3j..50+3j (row = d0*16+d1*4+d2),
    two per kt slot (rows 0:64 / 64:128) for row-tiled matmul pairs;
    feature 63 rides in the third pair's B half (rows 64:68).  Columns are
    u-major (col = u*32 + r).
    """
    f16 = np.float16
    FT = 4 * NQ                      # first triple feature
    kt_cores, xt_cores = [], []
    for bi in range(BSH):
        Xc = X[bi * BLOC : (bi + 1) * BLOC]                    # [512, 64, 4]
        for uj in range(USH):
            Ku = K[:, :, :, uj * ULOC : (uj + 1) * ULOC]       # [4,32,64,32]
            Kf = np.ascontiguousarray(
                Ku.transpose(2, 0, 3, 1).reshape(F, D, RUL)
            )                                                   # [f, d, col]
            kt = np.zeros((NSLOT, 128, RUL), dtype=f16)
            xt = np.zeros((NBT, 128, NSLOT * 128), dtype=f16)

            def put_x(slot, rows, arr):  # arr [BLOC, nrows]
                for t in range(NBT):
                    xt[t, rows, slot * 128 : (slot + 1) * 128] = arr[
                        t * 128 : (t + 1) * 128
                    ].T

            # triple pairs in slots 0..NPAIR-1
            for p in range(NPAIR):
                for s in range(2):
                    j = 2 * p + s
                    rows = slice(64 * s, 64 * s + 64)
                    f0 = FT + 3 * j
                    K3 = (
                        Kf[f0][:, None, None, :]
                        * Kf[f0 + 1][None, :, None, :]
                        * Kf[f0 + 2][None, None, :, :]
                    ).reshape(64, RUL)
                    X3 = (
                        Xc[:, f0, :, None, None]
                        * Xc[:, f0 + 1, None, :, None]
                        * Xc[:, f0 + 2, None, None, :]
                    ).reshape(BLOC, 64)
                    kt[p, rows] = K3
                    put_x(p, rows, X3)
            # quads in slots NPAIR + 2q + h
            for q in range(NQ):
                f0 = 4 * q
                K012 = (
                    Kf[f0][:, None, None, :]
                    * Kf[f0 + 1][None, :, None, :]
                    * Kf[f0 + 2][None, None, :, :]
                ).reshape(64, RUL)
                X012 = (
                    Xc[:, f0, :, None, None]
                    * Xc[:, f0 + 1, None, :, None]
                    * Xc[:, f0 + 2, None, None, :]
                ).reshape(BLOC, 64)
                for h in range(2):
                    slot = NPAIR + 2 * q + h
                    kt[slot] = (
                        K012[:, None, :] * Kf[f0 + 3][2 * h : 2 * h + 2][None, :, :]
                    ).reshape(128, RUL)
                    X4h = (
                        X012[:, :, None]
                        * Xc[:, f0 + 3, 2 * h : 2 * h + 2][:, None, :]
                    ).reshape(BLOC, 128)
                    put_x(slot, slice(0, 128), X4h)
            kt_cores.append(np.ascontiguousarray(kt))
            xt_cores.append(np.ascontiguousarray(xt))
    return [{"xt": xt_cores[c], "kt": kt_cores[c]} for c in range(NCORES)]


def kernel(**inputs):
    from concourse.bass_utils import run_bass_kernel_spmd

    X = np.asarray(inputs["X"], dtype=np.float32)
    K = np.asarray(inputs["kernel"], dtype=np.float32)
    assert X.shape == (B, F, D) and K.shape == (D, R, F, U)

    if "nc" not in _cached:
        _cached["nc"] = _build_nc()
    nc = _cached["nc"]

    in_maps = _host_prep(X, K)
    res = run_bass_kernel_spmd(nc, in_maps, core_ids=list(range(NCORES)))
    out = np.zeros((B, U), dtype=np.float32)
    for c in range(NCORES):
        bi, uj = divmod(c, USH)
        out[bi * BLOC : (bi + 1) * BLOC, uj * ULOC : (uj + 1) * ULOC] = res.results[
            c
        ]["out"]
    return out


# revision 20
# speedup vs baseline: 1.0527x; 1.0021x over previous
"""Trainium2 Bass kernel for nn_CP_Based (CP-decomposition interaction layer).

Math (full problem):
    t[b,f,r,u] = sum_d X[b,f,d] * K[d,r,f,u]      (B=1024, F=64, D=4, R=32, U=128)
    had[b,r,u] = prod_f t[b,f,r,u]
    out[b,u]   = sum_r had[b,r,u]

Strategy (v6):
  * Shard batch x units across 8 cores as (2 batch halves) x (4 unit
    quarters): per core B_loc=512 (4 partition tiles of 128) and RU_loc =
    32r x 32u = 1024 columns (u-major, r contiguous for the final reduce).
  * Host-side feature grouping: 16 triples (K=64, row-group pairs sharing a
    kt slot) + 4 quads (K=256 as 2 PSUM-accumulated K=128 passes) = 20
    factor tiles [128,1024] per batch tile.  Matmul inputs fp16.
  * PSUM ring: narrow [128,1024] tiles, bufs=4 (8 banks).  ALL consumers are
    narrow single-tile ops so every bank releases independently at ~PE pace
    (wide 2-slot rings serialize mm->consume and pace the whole btile).
  * Consumers: 7 tiles (incl. ALL 4 quads - numerically the touchiest) fold
    into a narrow fp32 fused chain G on DVE (one PSUM operand per op, no
    16-bit quantization).  13 tiles drain via Act ACTIVATE into bf16, packed
    pairwise into [128,2048] wide buffers so the product tree runs as wide
    bf16 muls in the DVE 2x tensor_tensor mode (~1.22us/2048 cols).  bf16,
    not fp16: DVE's fast 16-bit uop programs are bf16-only.
  * Tree: DVE chains 3 wide bufs, Pool chains the other 3 (2 wide muls),
    DVE combines + folds + handles the leftover narrow tile; the final
    merge with G runs on Pool and the strided r-reduce on DVE.  Cross-btile
    tail ops ride DVE's idle ramp of the next btile; finalization is
    deferred one btile (emitted mid-btile so queues never head-of-line
    block).
  * Input DMA split across two DGE rings (sync + gpsimd) in consumption
    order so the first matmul starts ~2.5us in.
"""

import numpy as np

B, F, D, R, U = 1024, 64, 4, 32, 128
NCORES = 8
BSH, USH = 2, 4                 # batch shards x unit shards
BLOC = B // BSH                 # 512 batch rows per core
NBT = BLOC // 128               # 4 batch tiles of 128
ULOC = U // USH                 # 32 units per core
RUL = R * ULOC                  # 1024 columns (u-major: col = u*32 + r)
NQ = 4                          # quads (features 0..15)
NT = 16                         # triples (features 16..63)
NTILE = NT + NQ                 # 20 factor tiles per batch tile
NPAIR = NT // 2                 # triple pairs (kt slots 0..7)
NSLOT = NPAIR + 2 * NQ          # kt slots: 8 triple-pairs + 2 per quad

FUSED = (2, 5, 8, 16, 17, 18, 19)   # DVE fp32 fused chain (all quads anchored)
ACTS = tuple(i for i in range(NTILE) if i not in FUSED)  # 13 Act drains
# pack consecutive Act tiles into halves of wide bf16 buffers
APAIR = tuple((ACTS[2 * k], ACTS[2 * k + 1]) for k in range(len(ACTS) // 2))
ALONE = ACTS[-1]                # leftover narrow tile (15)
# production order: quads interleaved mid-btile so their serial DVE G-ops
# land in DVE's slack window; btile tail is all fast-release Act drains
PORDER = (0, 1, 2, 3, 16, 4, 5, 17, 6, 7, 18, 8, 9, 19, 10, 11, 12, 13, 14, 15)

_cached = {}


def _build_nc():
    import concourse.bass as bass
    import concourse.mybir as mybir
    import concourse.tile as tile
    from concourse import bacc

    fp32 = mybir.dt.float32
    fp16 = mybir.dt.float16
    bf16 = mybir.dt.bfloat16
    nc = bacc.Bacc("TRN2", target_bir_lowering=False, debug=False)

    xt_d = nc.dram_tensor("xt", [NBT, 128, NSLOT * 128], fp16, kind="ExternalInput").ap()
    kt_d = nc.dram_tensor("kt", [NSLOT, 128, RUL], fp16, kind="ExternalInput").ap()
    out_d = nc.dram_tensor("out", [BLOC, ULOC], fp32, kind="ExternalOutput").ap()

    W2 = 2 * RUL

    # map act tile -> (pair index, half) or None for the leftover
    half_of = {}
    for k, (i, j) in enumerate(APAIR):
        half_of[i] = (k, 0)
        half_of[j] = (k, 1)

    with tile.TileContext(nc) as tc:
        with (
            tc.tile_pool(name="kt", bufs=1) as ktpool,
            tc.tile_pool(name="xt", bufs=1) as xtpool,
            tc.tile_pool(name="fb", bufs=12) as fbpool,
            tc.tile_pool(name="fn", bufs=2) as fnpool,
            tc.tile_pool(name="gg", bufs=2) as ggpool,
            tc.tile_pool(name="tt", bufs=2) as ttpool,
            tc.tile_pool(name="uu", bufs=2) as uupool,
            tc.tile_pool(name="vv", bufs=2) as vvpool,
            tc.tile_pool(name="oo", bufs=2) as oopool,
            tc.tile_pool(name="out", bufs=2) as outpool,
            tc.tile_pool(name="ps", bufs=4, space="PSUM") as pspool,
        ):
            xts = [
                xtpool.tile([128, NSLOT * 128], fp16, tag=f"xt{t}", name=f"xt{t}")
                for t in range(NBT)
            ]
            kts = [
                ktpool.tile([128, RUL], fp16, tag=f"kt{s}", name=f"kt{s}")
                for s in range(NSLOT)
            ]
            # three DGE rings: sync + scalar (HW) and gpsimd (SW).  First
            # matmul needs xt0 cols 0:128 + kt0; split xt0 into consumption-
            # ordered column chunks and fan the urgent transfers out.
            # kt slots ordered by first use under PORDER; three DGE rings
            nc.scalar.dma_start(kts[0][:], kt_d[0])
            nc.sync.dma_start(xts[0][:], xt_d[0])
            for s in (1, 9, 2, 11, 3, 13, 4, 15):
                nc.gpsimd.dma_start(kts[s][:], kt_d[s])
            for s in (8, 10, 12, 14, 5, 6, 7):
                nc.sync.dma_start(kts[s][:], kt_d[s])
            for t in range(1, NBT):
                nc.sync.dma_start(xts[t][:], xt_d[t])

            pending = []

            def xsl(s):
                return slice(s * 128, (s + 1) * 128)

            for t in range(NBT):
                xt = xts[t]

                Gb = [
                    ggpool.tile([128, RUL], fp32, tag=f"G{i}", name=f"G{i}")
                    for i in range(2)
                ]
                Tb = [
                    ttpool.tile([128, W2], bf16, tag=f"T{i}", name=f"T{i}")
                    for i in range(2)
                ]
                Ub = [
                    uupool.tile([128, W2], bf16, tag=f"U{i}", name=f"U{i}")
                    for i in range(2)
                ]
                Vw = vvpool.tile([128, W2], bf16, tag="V", name="V")
                Sn = vvpool.tile([128, RUL], bf16, tag="Sn", name="Sn")
                Sn2 = vvpool.tile([128, RUL], bf16, tag="Sn2", name="Sn2")
                fbs = [
                    fbpool.tile([128, W2], bf16, tag="fb", name="fb")
                    for _ in range(len(APAIR))
                ]
                fnn = fnpool.tile([128, RUL], bf16, tag="fn", name="fn")

                ng = 0
                for i in PORDER:
                    ps = pspool.tile([128, RUL], fp32, tag="ps", name="ps")
                    if i < NT:  # triple half: pair p = i//2, sub s = i%2
                        p, s = divmod(i, 2)
                        rows = slice(64 * s, 64 * s + 64)
                        for h in range(2):
                            cs = slice(512 * h, 512 * h + 512)
                            nc.tensor.matmul(
                                ps[:, cs],
                                xt[rows, xsl(p)],
                                kts[p][rows, cs],
                                start=True,
                                stop=True,
                                tile_position=(64 * s, 0),
                            )
                    else:  # quad: 2 psum-accumulated K=128 passes
                        q = i - NT
                        for h in range(2):
                            slot = NPAIR + 2 * q + h
                            for c in range(2):
                                cs = slice(512 * c, 512 * c + 512)
                                nc.tensor.matmul(
                                    ps[:, cs],
                                    xt[:, xsl(slot)],
                                    kts[slot][:, cs],
                                    start=(h == 0),
                                    stop=(h == 1),
                                )

                    if i in FUSED:
                        if ng == 0:
                            nc.vector.tensor_copy(Gb[0][:], ps[:])
                        else:
                            nc.vector.tensor_mul(
                                Gb[ng % 2][:], Gb[(ng - 1) % 2][:], ps[:]
                            )
                        ng += 1
                    elif i == ALONE:
                        nc.scalar.copy(fnn[:], ps[:])
                        nc.vector.tensor_mul(Sn2[:], Sn[:], fnn[:])
                    else:
                        k, h = half_of[i]
                        nc.scalar.copy(fbs[k][:, h * RUL : (h + 1) * RUL], ps[:])
                        # Pool (slow, 4.1us/wide) gets the EARLY buffers so it
                        # finishes within the btile; DVE (fast) gets the LATE
                        # ones so the cross-btile tail is short.
                        if i == APAIR[1][1]:       # B0,B1 drained
                            nc.gpsimd.tensor_mul(Ub[0][:], fbs[0][:], fbs[1][:])
                        elif i == APAIR[2][1]:     # B2 drained
                            nc.gpsimd.tensor_mul(Ub[1][:], Ub[0][:], fbs[2][:])
                        elif i == APAIR[4][1]:     # B3,B4 drained
                            nc.vector.tensor_mul(Tb[0][:], fbs[3][:], fbs[4][:])
                        elif i == APAIR[5][1]:     # B5 drained
                            nc.vector.tensor_mul(Tb[1][:], Tb[0][:], fbs[5][:])

                    if i == 2 and pending:
                        pending.pop(0)()

                    # tail right after T1 (Sn2 follows at the ALONE drain,
                    # which is the last position)
                    if i == APAIR[5][1]:
                        nc.vector.tensor_mul(Vw[:], Tb[1][:], Ub[1][:])
                        nc.vector.tensor_mul(Sn[:], Vw[:, 0:RUL], Vw[:, RUL:W2])

                def finalize(t=t, G=Gb[(len(FUSED) - 1) % 2], Sn2=Sn2):
                    O = oopool.tile([128, RUL], fp32, tag="O", name="O")
                    osum = outpool.tile([128, ULOC], fp32, tag="osum", name="osum")
                    nc.gpsimd.tensor_mul(O[:], G[:], Sn2[:])
                    nc.vector.tensor_reduce(
                        osum[:],
                        O[:].rearrange("p (u r) -> p u r", r=R),
                        axis=mybir.AxisListType.X,
                        op=mybir.AluOpType.add,
                    )
                    nc.sync.dma_start(out_d[t * 128 : (t + 1) * 128, :], osum[:])

                pending.append(finalize)

            for fin in pending:
                fin()

    nc.compile()
    return nc


def _host_prep(X, K):
    """Repack inputs into per-core fp16 stationary/moving operands.

    Quad q covers features 4q..4q+3 as two K=128 PSUM-accumulated passes
    (row = ((d0*4+d1)*4+d2)*2 + l, l indexing half of the 4th feature's
    d range).  Triples cover features 48+3j..50+3j (row = d0*16+d1*4+d2),
    two per kt slot (rows 0:64 / 64:128) for row-tiled matmul pairs;
    feature 63 rides in the third pair's B half (rows 64:68).  Columns are
    u-major (col = u*32 + r).
    """
    f16 = np.float16
    FT = 4 * NQ                      # first triple feature
    kt_cores, xt_cores = [], []
    for bi in range(BSH):
        Xc = X[bi * BLOC : (bi + 1) * BLOC]                    # [512, 64, 4]
        for uj in range(USH):
            Ku = K[:, :, :, uj * ULOC : (uj + 1) * ULOC]       # [4,32,64,32]
            Kf = np.ascontiguousarray(
                Ku.transpose(2, 0, 3, 1).reshape(F, D, RUL)
            )                                                   # [f, d, col]
            kt = np.zeros((NSLOT, 128, RUL), dtype=f16)
            xt = np.zeros((NBT, 128, NSLOT * 128), dtype=f16)

            def put_x(slot, rows, arr):  # arr [BLOC, nrows]
                for t in range(NBT):
                    xt[t, rows, slot * 128 : (slot + 1) * 128] = arr[
                        t * 128 : (t + 1) * 128
                    ].T

            # triple pairs in slots 0..NPAIR-1
            for p in range(NPAIR):
                for s in range(2):
                    j = 2 * p + s
                    rows = slice(64 * s, 64 * s + 64)
                    f0 = FT + 3 * j
                    K3 = (
                        Kf[f0][:, None, None, :]
                        * Kf[f0 + 1][None, :, None, :]
                        * Kf[f0 + 2][None, None, :, :]
                    ).reshape(64, RUL)
                    X3 = (
                        Xc[:, f0, :, None, None]
                        * Xc[:, f0 + 1, None, :, None]
                        * Xc[:, f0 + 2, None, None, :]
                    ).reshape(BLOC, 64)
                    kt[p, rows] = K3
                    put_x(p, rows, X3)
            # quads in slots NPAIR + 2q + h
            for q in range(NQ):
                f0 = 4 * q
                K012 = (
                    Kf[f0][:, None, None, :]
                    * Kf[f0 + 1][None, :, None, :]
                    * Kf[f0 + 2][None, None, :, :]
                ).reshape(64, RUL)
                X012 = (
                    Xc[:, f0, :, None, None]
                    * Xc[:, f0 + 1, None, :, None]
                    * Xc[:, f0 + 2, None, None, :]
                ).reshape(BLOC, 64)
                for h in range(2):
                    slot = NPAIR + 2 * q + h
                    kt[slot] = (
                        K012[:, None, :] * Kf[f0 + 3][2 * h : 2 * h + 2][None, :, :]
                    ).reshape(128, RUL)
                    X4h = (
                        X012[:, :, None]
                        * Xc[:, f0 + 3, 2 * h : 2 * h + 2][:, None, :]
                    ).reshape(BLOC, 128)
                    put_x(slot, slice(0, 128), X4h)
            kt_cores.append(np.ascontiguousarray(kt))
            xt_cores.append(np.ascontiguousarray(xt))
    return [{"xt": xt_cores[c], "kt": kt_cores[c]} for c in range(NCORES)]


def kernel(**inputs):
    from concourse.bass_utils import run_bass_kernel_spmd

    X = np.asarray(inputs["X"], dtype=np.float32)
    K = np.asarray(inputs["kernel"], dtype=np.float32)
    assert X.shape == (B, F, D) and K.shape == (D, R, F, U)

    if "nc" not in _cached:
        _cached["nc"] = _build_nc()
    nc = _cached["nc"]

    in_maps = _host_prep(X, K)
    res = run_bass_kernel_spmd(nc, in_maps, core_ids=list(range(NCORES)))
    out = np.zeros((B, U), dtype=np.float32)
    for c in range(NCORES):
        bi, uj = divmod(c, USH)
        out[bi * BLOC : (bi + 1) * BLOC, uj * ULOC : (uj + 1) * ULOC] = res.results[
            c
        ]["out"]
    return out
